# revision 13
# baseline (speedup 1.0000x reference)
"""DAP (PixelShuffle(2) + AvgPool2d(2,2)) == channel-group mean, on 8 TRN2 cores.

Full input x[16, 128, 256, 256] f32 -> out[16, 32, 256, 256] where
out[b, c] = mean(x[b, 4c:4c+4]) over the 4-channel group.

Sharding: data-parallel over batch; core i processes x[2i:2i+2].

Per-core bass kernel (x_loc [2, 128, 256, 256]):
  view x_loc as [b, G=8 superblocks, p=128 partitions, cc=16 chans, e=512]
  where the spatial plane (65536 elems) is split p*512+e, so every DMA moves
  2KB contiguous runs into/out of 128 partitions.
  For each (b, G): one 4 MiB load -> 2 DVE adds (group-of-4 reduction)
  -> ACT scale 0.25 -> one 1 MiB store.
"""

import numpy as np

import concourse.bass as bass
import concourse.mybir as mybir
import concourse.tile as tile
from concourse import bacc
from concourse.bass_utils import run_bass_kernel_spmd

N_CORES = 8
B_FULL, C_IN, H, W = 16, 128, 256, 256
K = 2
C_OUT = C_IN // (K * K)  # 32
B_LOC = B_FULL // N_CORES  # 2
S = H * W  # 65536 spatial
E = 512  # elems per partition-row chunk (2KB)
P = 128  # partitions
CC = 16  # channels per superblock (4 output groups)
G_BLOCKS = C_IN // CC  # 8 superblocks per batch

_cache = {}


def _build_nc(
    repeat: int = 1,
    hw_loop: int = 0,
    cc: int = CC,
    in_eng: str = "sync",
    out_eng: str = "sync",
    bufs_in: int = 3,
    bufs_mid: int = 3,
    bufs_out: int = 3,
    fuse: bool = False,
    split_in: int = 1,
    contig_in: bool = False,
    contig_out: bool = False,
    flat_dve: bool = False,
    dve3d: bool = False,
    mul_eng: str = "scalar",
    alt_rings: bool = False,
):
    nc = bacc.Bacc("TRN2", target_bir_lowering=False, debug=False)
    x = nc.dram_tensor("x", [B_LOC, C_IN, H, W], mybir.dt.float32, kind="ExternalInput")
    y = nc.dram_tensor(
        "y", [B_LOC, C_OUT, H, W], mybir.dt.float32, kind="ExternalOutput"
    )
    # [b, c, h, w] -> [b, G, p, cc, e]: channel = cc*G + cc, spatial = p*512 + e
    x_sb = (
        x.ap()
        .rearrange("b c h w -> b c (h w)")
        .rearrange("b (G cc) (p e) -> b G p cc e", cc=cc, e=E)
    )
    # [b, c, h, w] -> [b, G, p, g, e]: out channel = (cc//4)*G + g
    y_sb = (
        y.ap()
        .rearrange("b c h w -> b c (h w)")
        .rearrange("b (G g) (p e) -> b G p g e", g=cc // 4, e=E)
    )
    g_blocks = C_IN // cc
    engs = {"sync": nc.sync, "scalar": nc.scalar, "gpsimd": nc.gpsimd}
    ein, eout = engs[in_eng], engs[out_eng]
    # timing-only contiguous views (numerically WRONG, same bytes/shape)
    x_ct = x.ap().rearrange("b c h w -> (b c h w)").rearrange(
        "(n p j) -> n p j", p=P, j=cc * E
    )
    y_ct = y.ap().rearrange("b c h w -> (b c h w)").rearrange(
        "(n p j) -> n p j", p=P, j=(cc // 4) * E
    )

    n_g = cc // 4  # output channels per superblock
    with tile.TileContext(nc) as tc:
        with (
            tc.tile_pool(name="inp", bufs=bufs_in) as inp,
            tc.tile_pool(name="mid", bufs=bufs_mid) as mid,
            tc.tile_pool(name="outp", bufs=bufs_out) as outp,
            tc.tile_pool(name="dummy", bufs=2) as dummyp,
        ):

            def one_pass_flat():
                # load with channels permuted (c g e) so every DVE op is 2D
                assert cc == 16
                for b in range(B_LOC):
                    for G in range(g_blocks):
                        t = inp.tile([P, 4, 4, E], mybir.dt.float32)
                        xg = x_sb[b, G].rearrange("p (g c) e -> p c g e", g=4)
                        for c in range(4):
                            ein.dma_start(out=t[:, c], in_=xg[:, c])
                        flat = t.rearrange("p c g e -> p (c g e)")
                        v = mid.tile([P, 2 * 4 * E], mybir.dt.float32)
                        nc.vector.tensor_add(
                            out=v[:], in0=flat[:, 0 : 4 * 4 * E // 2],
                            in1=flat[:, 4 * 4 * E // 2 : 4 * 4 * E],
                        )
                        o = outp.tile([P, 4 * E], mybir.dt.float32)
                        nc.vector.tensor_add(
                            out=o[:], in0=v[:, 0 : 4 * E], in1=v[:, 4 * E : 8 * E]
                        )
                        nc.scalar.mul(o[:], o[:], 0.25)
                        eout.dma_start(
                            out=y_sb[b, G], in_=o.rearrange("p (g e) -> p g e", e=E)
                        )

            def one_pass():
                if flat_dve:
                    one_pass_flat()
                    return
                for b in range(B_LOC):
                    for G in range(g_blocks):
                        t = inp.tile([P, cc, E], mybir.dt.float32)
                        if alt_rings:
                            ein_i = engs["sync" if (G % 2 == 0) else "scalar"]
                            eout_i = engs["scalar" if (G % 2 == 0) else "sync"]
                        else:
                            ein_i, eout_i = ein, eout
                        if contig_in:
                            ein.dma_start(
                                out=t.rearrange("p c e -> p (c e)"),
                                in_=x_ct[b * g_blocks + G],
                            )
                        elif split_in == 1:
                            ein_i.dma_start(out=t[:], in_=x_sb[b, G])
                        else:
                            h = cc // split_in
                            es = [ein, eout] if split_in == 2 else [ein] * split_in
                            for s in range(split_in):
                                es[s % len(es)].dma_start(
                                    out=t[:, s * h : (s + 1) * h, :],
                                    in_=x_sb[b, G, :, s * h : (s + 1) * h, :],
                                )
                        t4 = t.rearrange("p (g c) e -> p g c e", g=n_g)
                        w = mid.tile([P, n_g, 2, E], mybir.dt.float32)
                        if dve3d:
                            nc.vector.tensor_add(
                                out=w[:, :, 0, :], in0=t4[:, :, 0, :], in1=t4[:, :, 2, :]
                            )
                            nc.vector.tensor_add(
                                out=w[:, :, 1, :], in0=t4[:, :, 1, :], in1=t4[:, :, 3, :]
                            )
                        else:
                            nc.vector.tensor_add(
                                out=w[:], in0=t4[:, :, 0:2, :], in1=t4[:, :, 2:4, :]
                            )
                        o = outp.tile([P, n_g, E], mybir.dt.float32)
                        if mul_eng == "vector":
                            nc.vector.tensor_add(
                                out=o[:], in0=w[:, :, 0, :], in1=w[:, :, 1, :]
                            )
                            nc.vector.tensor_scalar_mul(o[:], o[:], 0.25)
                        elif fuse:
                            dm = dummyp.tile([P, 1], mybir.dt.float32)
                            nc.vector.tensor_tensor_reduce(
                                out=o[:],
                                in0=w[:, :, 0, :],
                                in1=w[:, :, 1, :],
                                scale=0.25,
                                scalar=0.0,
                                op0=mybir.AluOpType.add,
                                op1=mybir.AluOpType.max,
                                accum_out=dm[:],
                            )
                        else:
                            nc.vector.tensor_add(
                                out=o[:], in0=w[:, :, 0, :], in1=w[:, :, 1, :]
                            )
                            nc.scalar.mul(o[:], o[:], 0.25)
                        if contig_out:
                            eout_i.dma_start(
                                out=y_ct[b * g_blocks + G],
                                in_=o.rearrange("p g e -> p (g e)"),
                            )
                        else:
                            eout_i.dma_start(out=y_sb[b, G], in_=o[:])

            if hw_loop:
                with tc.For_i(0, hw_loop, 1):
                    for _rep in range(repeat):
                        one_pass()
            else:
                for _rep in range(repeat):
                    one_pass()
    nc.compile()
    return nc


def kernel(x, kernel):
    k = int(kernel)
    assert k == K, f"kernel compiled for k=2, got {k}"
    x = np.asarray(x, dtype=np.float32)
    assert x.shape == (B_FULL, C_IN, H, W), x.shape

    if "nc" not in _cache:
        _cache["nc"] = _build_nc()
    nc = _cache["nc"]

    in_maps = [
        {"x": np.ascontiguousarray(x[i * B_LOC : (i + 1) * B_LOC])}
        for i in range(N_CORES)
    ]
    res = run_bass_kernel_spmd(nc, in_maps, core_ids=list(range(N_CORES)))
    _cache["last_results"] = res
    return np.concatenate([r["y"] for r in res.results], axis=0)


# revision 14
# speedup vs baseline: 1.0329x; 1.0329x over previous
"""DAP (PixelShuffle(2) + AvgPool2d(2,2)) == channel-group mean, on 8 TRN2 cores.

Full input x[16, 128, 256, 256] f32 -> out[16, 32, 256, 256] f32 where
out[b, c] = mean(x[b, 4c:4c+4, :, :]) over each 4-channel group.

Sharding: data-parallel over batch; core i processes x[2i:2i+2]. No
communication. Per-core traffic: 64 MiB read + 16 MiB written, so the kernel
is HBM-bound (~358 GB/s/core combined R+W -> ~234 us ideal).

Per-core bass program (x_loc [2, 128, 256, 256]):
  View x_loc as [b, G, p, cc, e] with G = 8 superblocks of cc = 16 channels,
  and the 65536-element spatial plane split as p*512 + e across p = 128
  partitions, so every DMA moves 2 KB contiguous runs into all 128 partitions.
  Per (b, G): one 4 MiB HWDGE load -> two full-width DVE adds reducing each
  group of 4 channels -> one ACT scale x0.25 -> one 1 MiB HWDGE store.
  DVE (~6.5 us) + ACT (~1.9 us) per 11.7 us superblock are fully hidden
  behind DMA; measured ~255 us/pass/core (~92-94% of the HBM roofline).
"""

import numpy as np

import concourse.mybir as mybir
import concourse.tile as tile
from concourse import bacc
from concourse.bass_utils import run_bass_kernel_spmd

N_CORES = 8
B_FULL, C_IN, H, W = 16, 128, 256, 256
K = 2
C_OUT = C_IN // (K * K)  # 32
B_LOC = B_FULL // N_CORES  # 2 batches per core
E = 512  # elements per partition-row chunk (2 KB)
P = 128  # SBUF partitions
CC = 16  # channels per superblock (4 output groups)
G_BLOCKS = C_IN // CC  # 8 superblocks per batch

_cache = {}


def _build_nc(repeat: int = 1, hw_loop: int = 0):
    """Build+compile the per-core program.

    repeat/hw_loop exist only for benchmarking (test.py): hw_loop wraps the
    pass in a For_i hardware loop, repeat unrolls passes inside the body.
    The production kernel uses the defaults (single pass, no loop).
    """
    nc = bacc.Bacc("TRN2", target_bir_lowering=False, debug=False)
    x = nc.dram_tensor("x", [B_LOC, C_IN, H, W], mybir.dt.float32, kind="ExternalInput")
    y = nc.dram_tensor(
        "y", [B_LOC, C_OUT, H, W], mybir.dt.float32, kind="ExternalOutput"
    )
    # [b, c, h, w] -> [b, G, p, cc, e]: channel = CC*G + cc, spatial = p*E + e
    x_sb = (
        x.ap()
        .rearrange("b c h w -> b c (h w)")
        .rearrange("b (G cc) (p e) -> b G p cc e", cc=CC, e=E)
    )
    # [b, c, h, w] -> [b, G, p, g, e]: out channel = (CC//4)*G + g
    y_sb = (
        y.ap()
        .rearrange("b c h w -> b c (h w)")
        .rearrange("b (G g) (p e) -> b G p g e", g=CC // 4, e=E)
    )

    n_g = CC // 4  # output channels per superblock
    with tile.TileContext(nc) as tc:
        with (
            tc.tile_pool(name="inp", bufs=3) as inp,
            tc.tile_pool(name="mid", bufs=3) as mid,
            tc.tile_pool(name="outp", bufs=3) as outp,
        ):

            def one_pass():
                for b in range(B_LOC):
                    for G in range(G_BLOCKS):
                        t = inp.tile([P, CC, E], mybir.dt.float32)
                        nc.sync.dma_start(out=t[:], in_=x_sb[b, G])
                        # rows of each group: cc = 4g + c, reduce over c
                        t4 = t.rearrange("p (g c) e -> p g c e", g=n_g)
                        w = mid.tile([P, n_g, 2, E], mybir.dt.float32)
                        nc.vector.tensor_add(
                            out=w[:], in0=t4[:, :, 0:2, :], in1=t4[:, :, 2:4, :]
                        )
                        o = outp.tile([P, n_g, E], mybir.dt.float32)
                        nc.vector.tensor_add(
                            out=o[:], in0=w[:, :, 0, :], in1=w[:, :, 1, :]
                        )
                        nc.scalar.mul(o[:], o[:], 0.25)
                        nc.sync.dma_start(out=y_sb[b, G], in_=o[:])

            if hw_loop:
                with tc.For_i(0, hw_loop, 1):
                    for _ in range(repeat):
                        one_pass()
            else:
                for _ in range(repeat):
                    one_pass()
    nc.compile()
    return nc


def kernel(x, kernel):
    k = int(kernel)
    assert k == K, f"kernel compiled for k=2, got {k}"
    x = np.asarray(x, dtype=np.float32)
    assert x.shape == (B_FULL, C_IN, H, W), x.shape

    if "nc" not in _cache:
        _cache["nc"] = _build_nc()
    nc = _cache["nc"]

    in_maps = [
        {"x": np.ascontiguousarray(x[i * B_LOC : (i + 1) * B_LOC])}
        for i in range(N_CORES)
    ]
    try:
        res = run_bass_kernel_spmd(nc, in_maps, core_ids=list(range(N_CORES)))
    except ModuleNotFoundError:
        # BASS_TRACE set in an environment without the axon NTFF hook;
        # rerun with tracing disabled.
        import os

        os.environ["BASS_NEVER_TRACE"] = "1"
        res = run_bass_kernel_spmd(nc, in_maps, core_ids=list(range(N_CORES)))
    _cache["last_results"] = res
    return np.concatenate([r["y"] for r in res.results], axis=0)
